# revision 1
# baseline (speedup 1.0000x reference)
"""Masked dot-product attention on 8 Trainium2 NeuronCores (Bass/Tile).

Problem: B=8, Nq=2048, Nk=2048, D=64 fp32; per-batch valid_lens L_b masks
keys k >= L_b before softmax (== excluding them).

Strategy: work-balanced SPMD. Key-sparsity is exploited by computing only
key chunks below ceil(L_b/128); to balance the uneven per-batch work, the
computation is decomposed into UNIFORM tasks: one task = (batch b, 512-query
group g, block of C=4 key chunks). Every core runs the identical program on
T tasks; the host packs each core's task inputs and combines the partial
softmax sums afterwards (numerators and denominators add across tasks,
since exp uses no max subtraction: |s|/8 <= ~6 keeps exp in range).

Per task (device):
  S^T[k, q] = K[k, :] . Q[q, :]       PE, f32r, row-packed 64-contraction
                                      matmul pairs (k chunks split across
                                      partition halves)
  E^T = exp(S^T / 8)                  ACT, fused scale, f32r out
  U[d', q] += V'[k, d']^T E^T[k, q]   PE, f32r; V' = [V | ones] with rows
                                      k >= L zeroed on host, so masked keys
                                      contribute 0 to numerator AND
                                      denominator; row 64 of U is the
                                      denominator.
Host: U[b, g] = sum of task partials; O = (U[:64] / U[64]).T

Numerics: f32r throughout the matmuls (~2e-4 rms end to end), exact exp on
ACT, fp32 accumulation in PSUM and on host.
"""
import math
import os
import sys

for _p in ("/opt/trn_rl_repo", "/root/.axon_site/_ro/trn_rl_repo"):
    if os.path.isdir(_p):
        if _p not in sys.path:
            sys.path.insert(0, _p)
        break

import ml_dtypes
import numpy as np

import bass_rust
import concourse.bass as bass
import concourse.tile as tile
from concourse import mybir
from concourse.bass_utils import run_bass_kernel_spmd
from concourse.tile_rust import add_dep_helper

F32 = mybir.dt.float32
F32R = mybir.dt.float32r
BF16 = mybir.dt.bfloat16

B, NQ, NK, D = 8, 2048, 2048, 64
NCHUNK = NK // 128          # 16 key chunks of 128
QG = 512                    # query-group width (matmul moving size)
NQG = NQ // QG
VW = D + 1                  # V' width (ones column)
C = 4                       # key chunks per task
NPAIR = C // 2              # row-packed matmul pairs per task

# Inputs per task: K2 [128, C*64] f32r, QT [64, QG] f32r (duplicated onto
# partitions 64-127 on-device), V' [128, C*66] bf16 (separate tensor: the
# f32r input DMA ROUNDS words to f32r precision, which would destroy
# bit-packed bf16 pairs).
VWP = VW + 1                        # V' chunk padded to 66 cols (4B alignment)
QP = 2                              # query groups per task
TK_K2 = C * 64                      # 256
TK_QT = QP * QG                     # 1024
TV_W = C * VWP                      # 264 bf16 per task
NV0 = 1                             # tasks covered by the early V DMA


def _split_waits(nc, maxw=1):
    """Walrus in this container rejects >1 sync wait on many instruction
    structs; hoist excess waits onto NoOps inserted just before."""
    cnt = 0
    for f in nc.m.functions:
        for bb in f.blocks:
            insts = bb.instructions
            i = 0
            while i < len(insts):
                ins = insts[i]
                si = ins.sync_info
                waits = list(si.on_wait) if si is not None and si.on_wait else []
                if len(waits) > maxw:
                    keep = waits[len(waits) - maxw:]
                    excess = waits[: len(waits) - maxw]
                    for j in range(0, len(excess), maxw):
                        cnt += 1
                        nop = mybir.InstNoOp(name=f"I-ws{cnt}", ins=[], outs=[])
                        nop.engine = ins.engine
                        nop.sync_info = bass_rust.SyncInfo(
                            on_wait=excess[j : j + maxw], on_update=[]
                        )
                        insts.insert(i, nop)
                        i += 1
                    ins.sync_info = bass_rust.SyncInfo(
                        on_wait=keep, on_update=list(si.on_update or [])
                    )
                i += 1
    return cnt


class _SlimTileContext(tile.TileContext):
    """Skip the exit sem-clears + double barrier (sems re-init at entry)."""

    def _drain_and_barrier(self, tick_clock, wait_clock):
        from concourse.vector_clock import ScopedClock
        drain_inst = self.nc.sync.drain()
        wait_clock.add_sem_waits(
            drain_inst.ins, ScopedClock({None: tick_clock.global_clock})
        )
        popped = self.nc._tile_sem_poison_stack.pop()
        assert popped is self._sem_poison


def _unbar_preamble(nc):
    """Strip the engine-boot barrier waits from follower engines in the
    entry block. The barrier only aligns engine boot (GpSimd is ~4us slow
    and is otherwise unused here: no SWDGE, and NRT re-inits semaphores per
    execution), so Sync can start input DMAs immediately."""
    bb0 = nc.m.functions[0].blocks[0]
    for ins in bb0.instructions:
        ty = type(ins).__name__
        if ty not in ("InstDrain", "InstEventSemaphore"):
            continue
        if str(ins.engine).endswith("Pool"):
            continue
        si = ins.sync_info
        if si is None or not si.on_wait:
            continue
        if all("barrier_" in w.ant_name for w in si.on_wait):
            ins.sync_info = bass_rust.SyncInfo(
                on_wait=[], on_update=list(si.on_update or [])
            )


_BUILT = {}


def _build(T):
    nc = bass.Bass(trn_type="TRN2")
    btk = nc.dram_tensor("btk", [128, T * TK_K2], F32R, kind="ExternalInput")
    btq = nc.dram_tensor("btq", [128, T * TK_QT], F32R, kind="ExternalInput")
    bv = nc.dram_tensor("bv", [128, T * TV_W], F32R, kind="ExternalInput")
    po = nc.dram_tensor("po", [T, VW, QP, QG], F32, kind="ExternalOutput")

    with _SlimTileContext(nc) as tc:
        with (
            tc.tile_pool(name="ipool", bufs=1) as ipool,
            tc.tile_pool(name="epool", bufs=6) as epool,
            tc.tile_pool(name="usb", bufs=3) as usb,
            tc.tile_pool(name="s2pool", bufs=2, space="PSUM") as s2pool,
            tc.tile_pool(name="upool", bufs=4, space="PSUM") as upool,
        ):
            # stages: 0 = task 0 (+V for tasks 0-1); 1 = tasks 1-3 (+V rest);
            # 2 = tasks 4..T-1. One bulk DMA per tensor per stage.
            nv1 = T - NV0
            tbv0 = ipool.tile([128, NV0 * C, VWP], F32R, tag="bv0")
            tbv1 = (ipool.tile([128, nv1 * C, VWP], F32R, tag="bv1",
                                name="tbv1")
                    if nv1 > 0 else None)

            def vchunk(t, ch):
                if t < NV0:
                    return tbv0[:, t * C + ch, 0:VW]
                return tbv1[:, (t - NV0) * C + ch, 0:VW]

            # PE warmup: dummy matmuls with no input deps run during the
            # input-DMA wait, so HAM reaches full clock before real work.
            wsb = ipool.tile([128, 640], BF16, tag="warm")
            nc.vector.memset(wsb[:], 0.0)
            wps = s2pool.tile([128, 2, QG], F32, tag="s2")
            for w in range(12):
                nc.tensor.matmul(wps[:, w % 2, :], wsb[:, 0:128],
                                 wsb[:, 128:640], start=True, stop=True)

            bounds = [0, 1, min(3, T), T]
            NS = 3
            tkq_tiles = []
            stage_dmas = [[] for _ in range(NS)]
            for s in range(NS):
                lo, hi = bounds[s], bounds[s + 1]
                n = hi - lo
                if n <= 0:
                    tkq_tiles.append(None)
                    continue
                tk = ipool.tile([128, n * TK_K2], F32R, tag=f"ks{s}")
                tq = ipool.tile([128, n * TK_QT], F32R, tag=f"qs{s}")
                stage_dmas[s].append(nc.sync.dma_start(
                    tk[:], btk[:, lo * TK_K2 : hi * TK_K2]))
                stage_dmas[s].append(nc.sync.dma_start(
                    tq[:], btq[:, lo * TK_QT : hi * TK_QT]))
                tkq_tiles.append((tk, tq, lo))
                if s == 0:
                    stage_dmas[0].append(nc.sync.dma_start(
                        tbv0[:], bv[:, 0 : NV0 * TV_W].rearrange(
                            "p (c w) -> p c w", w=VWP)))
                elif s == 1:
                    stage_dmas[1].append(nc.sync.dma_start(
                        tbv1[:], bv[:, NV0 * TV_W : T * TV_W].rearrange(
                            "p (c w) -> p c w", w=VWP)))
            for s in range(1, NS):
                if stage_dmas[s] and stage_dmas[s - 1]:
                    gate = stage_dmas[s - 1][-1]
                    for d in stage_dmas[s]:
                        add_dep_helper(d.ins, gate.ins, sync=True,
                                       reason=f"dma stage {s}")

            def task_aps(t, qh):
                s = 0
                for si in range(NS):
                    if bounds[si] <= t < bounds[si + 1]:
                        s = si
                        break
                tk, tq, lo = tkq_tiles[s]
                k2 = tk[:, (t - lo) * TK_K2 : (t - lo + 1) * TK_K2]
                qts_ = tq[:, (t - lo) * TK_QT + qh * QG :
                          (t - lo) * TK_QT + (qh + 1) * QG]
                return k2, qts_

            # software-pipelined: emit mm1/exp one iteration ahead of
            # mm2, so PE always has independent score work while ACT runs
            # the previous exp (PE never stalls on the exp result).
            iters = [(t, qh, pj) for t in range(T) for qh in range(QP)
                     for pj in range(NPAIR)]
            e2s = {}
            uts_t = {}
            ut_cur = [None]
            for i in range(len(iters) + 1):
                if i < len(iters):
                    t, qh, pj = iters[i]
                    k2, qtg = task_aps(t, qh)
                    ks = slice(pj * 128, (pj + 1) * 128)
                    s2 = s2pool.tile([128, 2, QG], F32, name=f"s2_{i}",
                                     tag="s2")
                    nc.tensor.matmul(s2[:, 0, :], k2[0:64, ks],
                                     qtg[0:64, :], start=True, stop=True,
                                     tile_position=(0, 0))
                    nc.tensor.matmul(s2[:, 1, :], k2[64:128, ks],
                                     qtg[64:128, :], start=True, stop=True,
                                     tile_position=(64, 0))
                    e2 = epool.tile([128, 2, QG], F32R, name=f"e2_{i}",
                                    tag="e2")
                    nc.scalar.activation(e2[:, :, :], s2[:, :, :],
                                         mybir.ActivationFunctionType.Exp,
                                         scale=0.125)
                    e2s[i] = e2
                if i > 0:
                    t, qh, pj = iters[i - 1]
                    e2 = e2s.pop(i - 1)
                    if pj == 0:
                        ut_cur[0] = upool.tile([VW, QG], F32,
                                               name=f"ut_{i}", tag="ut")
                    ut = ut_cur[0]
                    nc.tensor.matmul(ut[:], vchunk(t, 2 * pj), e2[:, 0, :],
                                     start=(pj == 0), stop=False)
                    nc.tensor.matmul(ut[:], vchunk(t, 2 * pj + 1),
                                     e2[:, 1, :], start=False,
                                     stop=(pj == NPAIR - 1))
                    if pj == NPAIR - 1:
                        if qh == 0:
                            uts_t[t] = usb.tile([VW, QP, QG], F32,
                                                name=f"uts_{t}", tag="uts")
                        uts = uts_t[t]
                        nc.vector.tensor_copy(uts[:, qh, :], ut[:])
                        nc.sync.dma_start(po[t][:, qh, :], uts[:, qh, :])

    _split_waits(nc)
    return nc


def _plan_tasks(valid_lens):
    """Tasks: (b, qp, c0): key chunks [c0, c0+C) of batch b, query groups
    [qp*QP, (qp+1)*QP)."""
    tasks = []
    for b in range(B):
        nch = max(1, math.ceil(int(valid_lens[b]) / 128))
        for qp in range(NQG // QP):
            for c0 in range(0, nch, C):
                tasks.append((b, qp, c0))
    while len(tasks) % B:
        tasks.append(None)
    return tasks


def _host_prep(queries, keys, values, valid_lens, tasks, T):
    queries = np.asarray(queries, dtype=np.float32)
    keys = np.asarray(keys, dtype=np.float32)
    values = np.asarray(values, dtype=np.float32)

    qts = queries.transpose(0, 2, 1)                     # [B, 64, 2048]
    kts = keys.transpose(0, 2, 1)                        # [B, 64, 2048]
    vps = np.zeros((B, NK + C * 128, VW), dtype=np.float32)
    for b in range(B):
        vps[b, :NK, :D] = values[b]
        vps[b, :NK, D] = 1.0
        vps[b, int(valid_lens[b]):NK, :] = 0.0           # mask keys >= L

    in_maps = []
    for c in range(B):
        kblob = np.zeros((128, T * TK_K2), dtype=np.float32)
        qblob = np.zeros((128, T * TK_QT), dtype=np.float32)
        vblob = np.zeros((128, T * TV_W), dtype=np.float32)
        for t in range(T):
            task = tasks[c * T + t]
            if task is None:
                continue
            b, g, c0 = task
            # K2: pairs of K^T chunks split across partition halves
            for pj in range(NPAIR):
                for half in range(2):
                    ch = c0 + 2 * pj + half
                    if ch < NCHUNK:
                        kblob[64 * half : 64 * half + 64,
                              t * TK_K2 + pj * 128 : t * TK_K2 + (pj + 1) * 128
                              ] = kts[b][:, ch * 128 : (ch + 1) * 128]
            qblob[0:64, t * TK_QT : (t + 1) * TK_QT] = (
                qts[b][:, g * QP * QG : (g + 1) * QP * QG])
            qblob[64:128, t * TK_QT : (t + 1) * TK_QT] = (
                qts[b][:, g * QP * QG : (g + 1) * QP * QG])
            # V' chunks (padded to 66 cols), chunk-packed
            vblk = np.zeros((C, 128, VWP), dtype=np.float32)
            vblk[:, :, :VW] = vps[b][c0 * 128 : (c0 + C) * 128, :].reshape(
                C, 128, VW)
            vblob[:, t * TV_W : (t + 1) * TV_W] = (
                vblk.transpose(1, 0, 2).reshape(128, TV_W))
        in_maps.append({"btk": kblob, "btq": qblob, "bv": vblob})
    return in_maps


def kernel(queries, keys, values, valid_lens):
    valid_lens = np.asarray(valid_lens)
    tasks = _plan_tasks(valid_lens)
    T = len(tasks) // B
    if T not in _BUILT:
        _BUILT[T] = _build(T)
    in_maps = _host_prep(queries, keys, values, valid_lens, tasks, T)
    res = run_bass_kernel_spmd(
        _BUILT[T],
        in_maps,
        core_ids=list(range(B)),
        trace=bool(os.environ.get("KERNEL_TRACE")),
    )
    kernel.last_result = res

    U = np.zeros((B, NQG, VW, QG), dtype=np.float32)
    for c in range(B):
        pc = np.asarray(res.results[c]["po"])  # [T, 65, QP, 512]
        for t in range(T):
            task = tasks[c * T + t]
            if task is None:
                continue
            b, g, _ = task
            for qh in range(QP):
                U[b, g * QP + qh] += pc[t, :, qh, :]
    out = np.empty((B, NQ, D), dtype=np.float32)
    for b in range(B):
        for g in range(NQG):
            out[b, g * QG : (g + 1) * QG, :] = (
                U[b, g, :D, :] / U[b, g, D : D + 1, :]
            ).T
    return out

